# revision 17
# baseline (speedup 1.0000x reference)
import numpy as np
import concourse.bass as bass
import concourse.bacc as bacc
import concourse.mybir as mybir
import concourse.tile as tile
from concourse.bass_utils import run_bass_kernel_spmd
from concourse.masks import make_identity

NCORES = 8
B = 64
GPC = 8          # graphs per core
IN = 5
H1, C1 = 4, 64
D1 = 256
C2 = 64

f32 = mybir.dt.float32
i32 = mybir.dt.int32
EXP = mybir.ActivationFunctionType.Exp
AX = mybir.AxisListType.X
ADD = mybir.AluOpType.add
MAXO = mybir.AluOpType.max
MULT = mybir.AluOpType.mult
EQ = mybir.AluOpType.is_equal
BYP = mybir.AluOpType.bypass

_cache = {}
_last = None  # (nc, in_maps) for re-run timing


def _build(N, G_PAD, NET, K, dbg=False):
    NLOC = GPC * G_PAD
    NT = NLOC // 128
    NPG = NCORES * NLOC
    E_PAD = NET * 128

    nc = bacc.Bacc("TRN2", target_bir_lowering=False, debug=False,
                   num_devices=NCORES)
    if dbg:
        x1_out = nc.declare_dram_parameter("x1_out", [NLOC, 256], f32,
                                           isOutput=True)
        x2_out = nc.declare_dram_parameter("x2_out", [NLOC, 64], f32,
                                           isOutput=True)
    xp = nc.declare_dram_parameter("xp", [N, 8], f32, isOutput=False)
    eidx = nc.declare_dram_parameter("eidx", [E_PAD, 2], i32, isOutput=False)
    ef = nc.declare_dram_parameter("ef", [E_PAD, 1], f32, isOutput=False)
    lgid = nc.declare_dram_parameter("lgid", [NLOC, 1], i32, isOutput=False)
    wlr1 = nc.declare_dram_parameter("wlr1", [16, 256], f32, isOutput=False)
    att1b = nc.declare_dram_parameter("att1b", [128, 256], f32, isOutput=False)
    bias1b = nc.declare_dram_parameter("bias1b", [128, 256], f32, isOutput=False)
    wlr2 = nc.declare_dram_parameter("wlr2", [256, 128], f32, isOutput=False)
    blr2b = nc.declare_dram_parameter("blr2b", [128, 128], f32, isOutput=False)
    att2b = nc.declare_dram_parameter("att2b", [128, 64], f32, isOutput=False)
    bias2b = nc.declare_dram_parameter("bias2b", [128, 64], f32, isOutput=False)
    wlin = nc.declare_dram_parameter("wlin", [192, 2], f32, isOutput=False)
    blinb = nc.declare_dram_parameter("blinb", [GPC, 2], f32, isOutput=False)
    iota = nc.declare_dram_parameter("iota", [128, 128], f32, isOutput=False)
    vmk = nc.declare_dram_parameter("vmk", [NLOC, 1], f32, isOutput=False)
    rcnt = nc.declare_dram_parameter("rcnt", [64, GPC], f32, isOutput=False)
    pooled = nc.declare_dram_parameter("pooled", [GPC, 2], f32, isOutput=True)

    with tile.TileContext(nc) as tc:
        with (
            tc.tile_pool(name="const", bufs=1) as cp,
            tc.tile_pool(name="wk", bufs=2) as pool,
            tc.tile_pool(name="ps", bufs=2, space="PSUM") as psS,
            tc.tile_pool(name="pagg", bufs=1, space="PSUM") as psA,
            tc.tile_pool(name="dio", bufs=1, space="DRAM") as dpool,
        ):
            # ---- constants ----
            wl1_t = cp.tile([8, 256], f32)
            nc.sync.dma_start(out=wl1_t[:], in_=wlr1[0:8, :])
            wr1_t = cp.tile([8, 256], f32)
            nc.sync.dma_start(out=wr1_t[:], in_=wlr1[8:16, :])
            att1b_t = cp.tile([128, 256], f32)
            nc.sync.dma_start(out=att1b_t[:], in_=att1b[:])
            bias1b_t = cp.tile([128, 256], f32)
            nc.sync.dma_start(out=bias1b_t[:], in_=bias1b[:])
            w2a_t = cp.tile([128, 128], f32)
            nc.sync.dma_start(out=w2a_t[:], in_=wlr2[0:128, :])
            w2b_t = cp.tile([128, 128], f32)
            nc.sync.dma_start(out=w2b_t[:], in_=wlr2[128:256, :])
            blr2b_t = cp.tile([128, 128], f32)
            nc.sync.dma_start(out=blr2b_t[:], in_=blr2b[:])
            att2b_t = cp.tile([128, 64], f32)
            nc.sync.dma_start(out=att2b_t[:], in_=att2b[:])
            bias2b_t = cp.tile([128, 64], f32)
            nc.sync.dma_start(out=bias2b_t[:], in_=bias2b[:])
            wlin_m = cp.tile([64, 2], f32)
            nc.sync.dma_start(out=wlin_m[:], in_=wlin[0:64, :])
            wlin_x = cp.tile([64, 2], f32)
            nc.sync.dma_start(out=wlin_x[:], in_=wlin[64:128, :])
            wlin_s = cp.tile([64, 2], f32)
            nc.sync.dma_start(out=wlin_s[:], in_=wlin[128:192, :])
            blinb_t = cp.tile([GPC, 2], f32)
            nc.sync.dma_start(out=blinb_t[:], in_=blinb[:])
            iota_t = cp.tile([128, 128], f32)
            nc.sync.dma_start(out=iota_t[:], in_=iota[:])
            rcnt_t = cp.tile([64, GPC], f32)
            nc.sync.dma_start(out=rcnt_t[:], in_=rcnt[:])
            ident = cp.tile([128, 128], f32)
            make_identity(nc, ident[:])
            eixAll = cp.tile([128, NET * 2], i32)
            nc.sync.dma_start(out=eixAll[:].rearrange("p (a j) -> p a j", j=2),
                              in_=eidx[:].rearrange("(a p) j -> p a j", p=128))
            efAll = cp.tile([128, NET], f32)
            nc.sync.dma_start(out=efAll[:].rearrange("p (a j) -> p a j", j=1),
                              in_=ef[:].rearrange("(a p) j -> p a j", p=128))
            vmAll = cp.tile([128, NT], f32)
            nc.sync.dma_start(out=vmAll[:].rearrange("p (a j) -> p a j", j=1),
                              in_=vmk[:].rearrange("(a p) j -> p a j", p=128))
            lgAll = cp.tile([128, NT], i32)
            nc.sync.dma_start(out=lgAll[:].rearrange("p (a j) -> p a j", j=1),
                              in_=lgid[:].rearrange("(a p) j -> p a j", p=128))
            xsT = cp.tile([64, NLOC], f32)
            xmT = cp.tile([64, NLOC], f32)
            xr2All = cp.tile([128, NT * 64], f32)

            ag_in = dpool.tile([NLOC, 64], f32)
            ag_out = dpool.tile([NPG, 64], f32)

            # ---- conv1 (edge-centric) + xl2/xr2 production ----
            et = 0
            for t in range(NT):
                # local node features + xr_loc = Wr1 @ x_loc + br1
                xpl = pool.tile([128, 8], f32, tag="xpl")
                nc.gpsimd.indirect_dma_start(
                    out=xpl[:], out_offset=None, in_=xp[:],
                    in_offset=bass.IndirectOffsetOnAxis(
                        ap=lgAll[:, t:t + 1], axis=0))
                tplP = psS.tile([8, 128], f32, tag="psA")
                nc.tensor.transpose(tplP[:], xpl[:], ident[:])
                xplT = pool.tile([8, 128], f32, tag="xplT")
                nc.scalar.copy(xplT[:], tplP[:])
                xrlP = psS.tile([128, 256], f32, tag="psC")
                nc.tensor.matmul(xrlP[:], xplT[:], wr1_t[:],
                                 start=True, stop=True)
                xrloc = pool.tile([128, 256], f32, tag="xrloc")
                nc.scalar.copy(xrloc[:], xrlP[:])

                aggP = psA.tile([128, 260], f32, tag="agg")
                for k in range(K[t]):
                    c0 = et * 2
                    xs = pool.tile([128, 8], f32, tag="xs")
                    nc.gpsimd.indirect_dma_start(
                        out=xs[:], out_offset=None, in_=xp[:],
                        in_offset=bass.IndirectOffsetOnAxis(
                            ap=eixAll[:, c0:c0 + 1], axis=0))
                    tpP = psS.tile([8, 128], f32, tag="psA")
                    nc.tensor.transpose(tpP[:], xs[:], ident[:])
                    xsT_e = pool.tile([8, 128], f32, tag="xsT_e")
                    nc.scalar.copy(xsT_e[:], tpP[:])
                    oh = pool.tile([128, 128], f32, tag="oh")
                    nc.vector.tensor_tensor(
                        out=oh[:],
                        in0=efAll[:, et:et + 1].to_broadcast((128, 128)),
                        in1=iota_t[:], op=EQ)
                    ohTP = psS.tile([128, 128], f32, tag="psA")
                    nc.tensor.transpose(ohTP[:], oh[:], ident[:])
                    ohT = pool.tile([128, 128], f32, tag="ohT")
                    nc.scalar.copy(ohT[:], ohTP[:])
                    # z = xl_src + xr_dst via accumulating matmuls
                    zP = psS.tile([128, 256], f32, tag="psB")
                    nc.tensor.matmul(zP[:], xsT_e[:], wl1_t[:],
                                     start=True, stop=False)
                    nc.tensor.matmul(zP[:], ohT[:], xrloc[:],
                                     start=False, stop=True)
                    zsb = pool.tile([128, 256], f32, tag="zsb")
                    nc.scalar.copy(zsb[:], zP[:])
                    z = pool.tile([128, 256], f32, tag="z")
                    nc.vector.scalar_tensor_tensor(
                        out=z[:], in0=zsb[:], scalar=0.2, in1=zsb[:],
                        op0=MULT, op1=MAXO)
                    zatt = pool.tile([128, 256], f32, tag="zatt")
                    nc.vector.tensor_mul(zatt[:], z[:], att1b_t[:])
                    lg = pool.tile([128, 4], f32, tag="lg")
                    nc.vector.tensor_reduce(
                        lg[:], zatt[:].rearrange("p (h c) -> p h c", c=C1),
                        AX, ADD)
                    w = pool.tile([128, 4], f32, tag="w")
                    nc.scalar.activation(w[:], lg[:], EXP)
                    xlP = psS.tile([128, 256], f32, tag="psC")
                    nc.tensor.matmul(xlP[:], xsT_e[:], wl1_t[:],
                                     start=True, stop=True)
                    rhs = pool.tile([128, 260], f32, tag="rhs")
                    nc.vector.tensor_mul(
                        rhs[:, 0:256].rearrange("p (h c) -> p h c", c=C1),
                        xlP[:].rearrange("p (h c) -> p h c", c=C1),
                        w[:].unsqueeze(2).to_broadcast((128, H1, C1)))
                    nc.scalar.copy(rhs[:, 256:260], w[:])
                    nc.tensor.matmul(aggP[:], oh[:], rhs[:],
                                     start=(k == 0), stop=(k == K[t] - 1))
                    et += 1
                agg = pool.tile([128, 260], f32, tag="agg_s")
                nc.scalar.copy(agg[:], aggP[:])
                sc = pool.tile([128, 4], f32, tag="sc")
                nc.vector.tensor_scalar_max(sc[:], agg[:, 256:260], 1e-30)
                r = pool.tile([128, 4], f32, tag="r")
                nc.vector.reciprocal(r[:], sc[:])
                h1 = pool.tile([128, 256], f32, tag="h1")
                nc.vector.tensor_mul(
                    h1[:].rearrange("p (h c) -> p h c", c=C1),
                    agg[:, 0:256].rearrange("p (h c) -> p h c", c=C1),
                    r[:].unsqueeze(2).to_broadcast((128, H1, C1)))
                nc.vector.tensor_add(h1[:], h1[:], bias1b_t[:])
                ng = pool.tile([128, 256], f32, tag="ng")
                nc.vector.tensor_scalar_min(ng[:], h1[:], 0.0)
                nc.scalar.activation(ng[:], ng[:], EXP)
                x1 = pool.tile([128, 256], f32, tag="x1")
                nc.vector.tensor_scalar_max(x1[:], h1[:], 0.0)
                nc.vector.tensor_add(x1[:], x1[:], ng[:])
                nc.vector.tensor_scalar_add(x1[:], x1[:], -1.0)
                if dbg:
                    nc.sync.dma_start(out=x1_out[t * 128:(t + 1) * 128, :],
                                      in_=x1[:])
                t1P = psS.tile([128, 128], f32, tag="psA")
                nc.tensor.transpose(t1P[:], x1[:, 0:128], ident[:])
                x1Ta = pool.tile([128, 128], f32, tag="x1Ta")
                nc.scalar.copy(x1Ta[:], t1P[:])
                t2P = psS.tile([128, 128], f32, tag="psB")
                nc.tensor.transpose(t2P[:], x1[:, 128:256], ident[:])
                x1Tb = pool.tile([128, 128], f32, tag="x1Tb")
                nc.scalar.copy(x1Tb[:], t2P[:])
                mmP = psS.tile([128, 128], f32, tag="psC")
                nc.tensor.matmul(mmP[:], x1Ta[:], w2a_t[:],
                                 start=True, stop=False)
                nc.tensor.matmul(mmP[:], x1Tb[:], w2b_t[:],
                                 start=False, stop=True)
                xlr2 = pool.tile([128, 128], f32, tag="xlr2")
                nc.vector.tensor_add(xlr2[:], mmP[:], blr2b_t[:])
                nc.gpsimd.dma_start(out=ag_in[t * 128:(t + 1) * 128, :],
                                    in_=xlr2[:, 0:64])
                nc.scalar.copy(xr2All[:, t * 64:(t + 1) * 64],
                               xlr2[:, 64:128])

            # ---- halo exchange: all-gather xl2 tables ----
            nc.gpsimd.collective_compute(
                "AllGather", BYP,
                replica_groups=[list(range(NCORES))],
                ins=[ag_in.opt()], outs=[ag_out.opt()])

            # ---- conv2 ----
            et = 0
            for t in range(NT):
                agg2P = psA.tile([128, 65], f32, tag="agg")
                for k in range(K[t]):
                    c0 = et * 2
                    xle = pool.tile([128, 64], f32, tag="xle")
                    nc.gpsimd.indirect_dma_start(
                        out=xle[:], out_offset=None, in_=ag_out[:],
                        in_offset=bass.IndirectOffsetOnAxis(
                            ap=eixAll[:, c0 + 1:c0 + 2], axis=0))
                    oh2 = pool.tile([128, 128], f32, tag="oh")
                    nc.vector.tensor_tensor(
                        out=oh2[:],
                        in0=efAll[:, et:et + 1].to_broadcast((128, 128)),
                        in1=iota_t[:], op=EQ)
                    ohT2P = psS.tile([128, 128], f32, tag="psA")
                    nc.tensor.transpose(ohT2P[:], oh2[:], ident[:])
                    ohT2 = pool.tile([128, 128], f32, tag="ohT")
                    nc.scalar.copy(ohT2[:], ohT2P[:])
                    xreP = psS.tile([128, 64], f32, tag="psC")
                    nc.tensor.matmul(xreP[:], ohT2[:],
                                     xr2All[:, t * 64:(t + 1) * 64],
                                     start=True, stop=True)
                    z2 = pool.tile([128, 64], f32, tag="z2")
                    nc.vector.tensor_add(z2[:], xle[:], xreP[:])
                    z2l = pool.tile([128, 64], f32, tag="z2l")
                    nc.vector.scalar_tensor_tensor(
                        out=z2l[:], in0=z2[:], scalar=0.2, in1=z2[:],
                        op0=MULT, op1=MAXO)
                    z2a = pool.tile([128, 64], f32, tag="z2a")
                    nc.vector.tensor_mul(z2a[:], z2l[:], att2b_t[:])
                    lg2 = pool.tile([128, 1], f32, tag="lg2")
                    nc.vector.tensor_reduce(lg2[:], z2a[:], AX, ADD)
                    w2 = pool.tile([128, 1], f32, tag="w2")
                    nc.scalar.activation(w2[:], lg2[:], EXP)
                    rhs2 = pool.tile([128, 65], f32, tag="rhs2")
                    nc.vector.tensor_mul(rhs2[:, 0:64], xle[:],
                                         w2[:].to_broadcast((128, 64)))
                    nc.scalar.copy(rhs2[:, 64:65], w2[:])
                    nc.tensor.matmul(agg2P[:], oh2[:], rhs2[:],
                                     start=(k == 0), stop=(k == K[t] - 1))
                    et += 1
                agg2 = pool.tile([128, 65], f32, tag="agg2s")
                nc.scalar.copy(agg2[:], agg2P[:])
                sc2 = pool.tile([128, 1], f32, tag="sc2")
                nc.vector.tensor_scalar_max(sc2[:], agg2[:, 64:65], 1e-30)
                r2 = pool.tile([128, 1], f32, tag="r2")
                nc.vector.reciprocal(r2[:], sc2[:])
                h2 = pool.tile([128, 64], f32, tag="h2")
                nc.vector.tensor_mul(h2[:], agg2[:, 0:64],
                                     r2[:].to_broadcast((128, 64)))
                nc.vector.tensor_add(h2[:], h2[:], bias2b_t[:])
                ng2 = pool.tile([128, 64], f32, tag="ng2")
                nc.vector.tensor_scalar_min(ng2[:], h2[:], 0.0)
                nc.scalar.activation(ng2[:], ng2[:], EXP)
                x2 = pool.tile([128, 64], f32, tag="x2")
                nc.vector.tensor_scalar_max(x2[:], h2[:], 0.0)
                nc.vector.tensor_add(x2[:], x2[:], ng2[:])
                nc.vector.tensor_scalar_add(x2[:], x2[:], -1.0)
                if dbg:
                    nc.sync.dma_start(out=x2_out[t * 128:(t + 1) * 128, :],
                                      in_=x2[:])
                x2s = pool.tile([128, 64], f32, tag="x2s")
                nc.vector.tensor_mul(x2s[:], x2[:],
                                     vmAll[:, t:t + 1].to_broadcast((128, 64)))
                bgp = pool.tile([128, 1], f32, tag="bgp")
                nc.vector.tensor_scalar(out=bgp[:], in0=vmAll[:, t:t + 1],
                                        scalar1=-1.0, scalar2=1e30,
                                        op0=ADD, op1=MULT)
                x2m = pool.tile([128, 64], f32, tag="x2m")
                nc.vector.tensor_add(x2m[:], x2s[:],
                                     bgp[:].to_broadcast((128, 64)))
                tsP = psS.tile([64, 128], f32, tag="psA")
                nc.tensor.transpose(tsP[:], x2s[:], ident[:])
                nc.scalar.copy(xsT[:, t * 128:(t + 1) * 128], tsP[:])
                tmP = psS.tile([64, 128], f32, tag="psB")
                nc.tensor.transpose(tmP[:], x2m[:], ident[:])
                nc.scalar.copy(xmT[:, t * 128:(t + 1) * 128], tmP[:])

            # ---- pooling + final linear ----
            smT = pool.tile([64, GPC], f32, tag="smT")
            nc.vector.tensor_reduce(
                smT[:], xsT[:].rearrange("p (g n) -> p g n", n=G_PAD), AX, ADD)
            mxT = pool.tile([64, GPC], f32, tag="mxT")
            nc.vector.tensor_reduce(
                mxT[:], xmT[:].rearrange("p (g n) -> p g n", n=G_PAD), AX, MAXO)
            mnT = pool.tile([64, GPC], f32, tag="mnT")
            nc.vector.tensor_mul(mnT[:], smT[:], rcnt_t[:])
            finP = psA.tile([GPC, 2], f32, tag="fin")
            nc.tensor.matmul(finP[:], mnT[:], wlin_m[:], start=True, stop=False)
            nc.tensor.matmul(finP[:], mxT[:], wlin_x[:], start=False, stop=False)
            nc.tensor.matmul(finP[:], smT[:], wlin_s[:], start=False, stop=True)
            outp = pool.tile([GPC, 2], f32, tag="outp")
            nc.vector.tensor_add(outp[:], finP[:], blinb_t[:])
            nc.sync.dma_start(out=pooled[:], in_=outp[:])

    nc.compile()
    return nc


def _prep(inputs):
    x = np.asarray(inputs["x"], np.float32)
    ei = np.asarray(inputs["edge_index"]).astype(np.int64)
    bt = np.asarray(inputs["batch"]).astype(np.int64)
    N = x.shape[0]

    sizes = np.bincount(bt, minlength=B).astype(np.int64)
    start = np.zeros(B, np.int64)
    start[1:] = np.cumsum(sizes)[:-1]
    G_PAD = int(np.ceil(max(int(sizes.max()), 1) / 128.0)) * 128
    NLOC = GPC * G_PAD
    NT = NLOC // 128

    rank = np.arange(N, dtype=np.int64) - start[bt]
    core_n = bt // GPC
    slot = bt % GPC
    loc = slot * G_PAD + rank
    pgid = core_n * NLOC + loc

    src = np.concatenate([ei[0], np.arange(N, dtype=np.int64)])
    dst = np.concatenate([ei[1], np.arange(N, dtype=np.int64)])
    ec = core_n[dst]
    dl = loc[dst]
    dt_ = dl // 128

    cnt = np.zeros((NCORES, NT), np.int64)
    for c in range(NCORES):
        cnt[c] = np.bincount(dt_[ec == c], minlength=NT)
    K = np.maximum(1, np.ceil(cnt.max(0) / 128.0)).astype(np.int64)
    toff = np.zeros(NT + 1, np.int64)
    toff[1:] = np.cumsum(K * 128)
    E_PAD = int(toff[-1])
    NET = E_PAD // 128

    eidx_np = np.zeros((NCORES, E_PAD, 2), np.int32)
    ef_np = np.full((NCORES, E_PAD, 1), -1.0, np.float32)
    for c in range(NCORES):
        m = np.nonzero(ec == c)[0]
        order = np.argsort(dl[m], kind="stable")
        me = m[order]
        dtile = dt_[me]
        grp_start = np.searchsorted(dtile, np.arange(NT), side="left")
        rpos = np.arange(len(me), dtype=np.int64) - grp_start[dtile]
        slots_ = toff[dtile] + rpos
        eidx_np[c, slots_, 0] = src[me].astype(np.int32)
        eidx_np[c, slots_, 1] = pgid[src[me]].astype(np.int32)
        ef_np[c, slots_, 0] = (dl[me] % 128).astype(np.float32)

    lgid_np = np.zeros((NCORES, NLOC, 1), np.int32)
    lgid_np[core_n, loc, 0] = np.arange(N, dtype=np.int32)

    xp_np = np.zeros((N, 8), np.float32)
    xp_np[:, :IN] = x
    xp_np[:, IN] = 1.0

    Wl1 = np.asarray(inputs["Wl1"], np.float32)
    bl1 = np.asarray(inputs["bl1"], np.float32)
    Wr1 = np.asarray(inputs["Wr1"], np.float32)
    br1 = np.asarray(inputs["br1"], np.float32)
    att1 = np.asarray(inputs["att1"], np.float32)
    bias1 = np.asarray(inputs["bias1"], np.float32)
    Wl2 = np.asarray(inputs["Wl2"], np.float32)
    bl2 = np.asarray(inputs["bl2"], np.float32)
    Wr2 = np.asarray(inputs["Wr2"], np.float32)
    br2 = np.asarray(inputs["br2"], np.float32)
    att2 = np.asarray(inputs["att2"], np.float32)
    bias2 = np.asarray(inputs["bias2"], np.float32)
    Wlin = np.asarray(inputs["Wlin"], np.float32)
    blin = np.asarray(inputs["blin"], np.float32)

    wlr1_np = np.zeros((16, 256), np.float32)
    wlr1_np[0:IN] = Wl1
    wlr1_np[IN] = bl1
    wlr1_np[8:8 + IN] = Wr1
    wlr1_np[8 + IN] = br1
    att1b_np = np.tile(att1.reshape(1, 256), (128, 1))
    bias1b_np = np.tile(bias1.reshape(1, 256), (128, 1))
    wlr2_np = np.concatenate([Wl2, Wr2], axis=1)
    blr2b_np = np.tile(np.concatenate([bl2, br2]).reshape(1, 128), (128, 1))
    att2b_np = np.tile(att2.reshape(1, 64), (128, 1))
    bias2b_np = np.tile(bias2.reshape(1, 64), (128, 1))
    iota_np = np.tile(np.arange(128, dtype=np.float32).reshape(1, 128), (128, 1))
    blinb_np = np.tile(blin.reshape(1, 2), (GPC, 1))

    vmk_np = np.zeros((NCORES, NLOC, 1), np.float32)
    rcnt_np = np.zeros((NCORES, 64, GPC), np.float32)
    for g in range(B):
        c, s = g // GPC, g % GPC
        vmk_np[c, s * G_PAD:s * G_PAD + int(sizes[g]), 0] = 1.0
        rcnt_np[c, :, s] = 1.0 / max(float(sizes[g]), 1.0)

    in_maps = []
    for c in range(NCORES):
        in_maps.append(dict(
            xp=xp_np, eidx=eidx_np[c], ef=ef_np[c], lgid=lgid_np[c],
            wlr1=wlr1_np, att1b=att1b_np, bias1b=bias1b_np,
            wlr2=wlr2_np, blr2b=blr2b_np, att2b=att2b_np, bias2b=bias2b_np,
            wlin=Wlin, blinb=blinb_np, iota=iota_np,
            vmk=vmk_np[c], rcnt=rcnt_np[c],
        ))
    return N, G_PAD, NET, tuple(int(v) for v in K), in_maps


def kernel(**inputs):
    global _last
    N, G_PAD, NET, K, in_maps = _prep(inputs)
    key = (N, G_PAD, NET, K)
    if key not in _cache:
        _cache[key] = _build(N, G_PAD, NET, K)
    nc = _cache[key]
    _last = (nc, in_maps)
    res = run_bass_kernel_spmd(nc, in_maps, list(range(NCORES)), trace=False)
    out = np.concatenate([res.results[c]["pooled"] for c in range(NCORES)],
                         axis=0)
    return out.astype(np.float32)


def _rerun():
    nc, in_maps = _last
    return run_bass_kernel_spmd(nc, in_maps, list(range(NCORES)), trace=False)
